# revision 1
# baseline (speedup 1.0000x reference)
"""BertCorrector kernel for 8 TRN2 NeuronCores.

Computes: segment-mean merge of subword encodings (sorted per-row segment
ids) followed by a dense vocab projection:
    merged[b,w,:] = mean_{s: ids[b,s]==w} enc[b,s,:]   (0 if empty)
    logits = merged @ W + b

Strategy: data-parallel over batch (4 samples/core).  The segment-mean is
computed on the TensorEngine as enc^T @ S where S is a per-sample one-hot
matrix pre-scaled by 1/count (built host-side from segment_ids).  That
directly yields merged TRANSPOSED ([H, W] chunks), which is exactly the
stationary-operand layout the vocab-projection matmul needs.  All matmul
inputs are bf16 (fp32 PSUM accumulation); the output is written f32.
"""

import numpy as np
import ml_dtypes

B, S, H = 32, 512, 768
V = 8192
WMAX = 256
NCORES = 8
PB = B // NCORES  # samples per core
P = 128

KC = S // P   # 4 token chunks (contraction of stage A)
KO = H // P   # 6 hidden chunks
WT = WMAX // P  # 2 word tiles
NV = 512      # vocab tile
NT = V // NV  # 16 vocab tiles

_compiled = None


def _build_program():
    import concourse.bass as bass
    import concourse.mybir as mybir
    from concourse import bacc
    from concourse.tile import TileContext

    bf16 = mybir.dt.bfloat16
    f32 = mybir.dt.float32

    nc = bacc.Bacc()
    enc_d = nc.dram_tensor("enc", [PB, S, H], bf16, kind="ExternalInput")
    aux_d = nc.dram_tensor("aux", [P, PB, 2, KC], f32, kind="ExternalInput")
    w_d = nc.dram_tensor("wmat", [H, V], bf16, kind="ExternalInput")
    out_d = nc.dram_tensor("out", [PB, WMAX, V], f32, kind="ExternalOutput")

    enc_r = enc_d.rearrange("b (kc p) h -> b p kc h", p=P)
    w_r = w_d.rearrange("(ko p) v -> p ko v", p=P)

    with TileContext(nc) as tc:
        with (
            tc.tile_pool(name="persist", bufs=1) as persist,
            tc.tile_pool(name="encp", bufs=2) as encp,
            tc.tile_pool(name="onehp", bufs=4) as onehp,
            tc.tile_pool(name="wp", bufs=3) as wp,
            tc.tile_pool(name="outp", bufs=8) as outp,
            tc.tile_pool(name="ps1", bufs=2, space="PSUM") as ps1,
            tc.tile_pool(name="ps2", bufs=6, space="PSUM") as ps2,
        ):
            # mergedT[h_in_chunk, ko, s, w] resident in SBUF (bf16)
            mergedT = persist.tile([P, KO, PB, WMAX], bf16)

            # Prefetch the first W chunks so stage B never stalls on them.
            w_tiles = {}

            def load_w(n):
                if n < NT:
                    t = wp.tile([P, KO, NV], bf16, tag="w")
                    nc.sync.dma_start(out=t[:], in_=w_r[:, :, n * NV:(n + 1) * NV])
                    w_tiles[n] = t

            # iota row (0..WMAX-1, identical on every partition), on-device
            iota_sb = persist.tile([P, WMAX], f32)
            nc.gpsimd.iota(
                iota_sb[:], pattern=[[1, WMAX]], base=0,
                channel_multiplier=0, allow_small_or_imprecise_dtypes=True,
            )
            # all samples' (segment id, 1/count) pairs in one contiguous DMA
            aux_sb = persist.tile([P, PB, 2, KC], f32)
            nc.sync.dma_start(out=aux_sb[:], in_=aux_d[:])

            # Warm the PE clock gate while the first input DMAs fly.
            warm_sb = persist.tile([P, P], bf16)
            nc.gpsimd.memset(warm_sb[:], 0.0)
            warm_ps = ps1.tile([P, 64], f32, tag="ps1")
            for _ in range(68):
                nc.tensor.matmul(
                    warm_ps[:], lhsT=warm_sb[:], rhs=warm_sb[:, :64],
                    start=True, stop=True,
                )

            # ---- Stage A: mergedT = enc^T @ scaled_onehot, per sample ----
            # Scaled one-hots are built on-chip for ALL samples up front
            # (DVE runs in emission order, so these never queue behind the
            # mergedT casts): oneh[tok,w] = (iota[w] == ids[tok]) / count
            oneh_tiles = []
            for s in range(PB):
                t = onehp.tile([P, KC, WMAX], bf16, tag="oneh", name=f"oneh{s}")
                for kc in range(KC):
                    nc.vector.tensor_scalar(
                        out=t[:, kc],
                        in0=iota_sb[:],
                        scalar1=aux_sb[:, s, 0, kc:kc + 1],
                        scalar2=aux_sb[:, s, 1, kc:kc + 1],
                        op0=mybir.AluOpType.is_equal,
                        op1=mybir.AluOpType.mult,
                    )
                oneh_tiles.append(t)

            for s in range(PB):
                enc_sb = encp.tile([P, KC, H], bf16, tag="enc")
                if s == 0:
                    # split across DMA queues so the first chunk lands sooner
                    for kc in range(KC):
                        nc.sync.dma_start(out=enc_sb[:, kc], in_=enc_r[s, :, kc])
                else:
                    nc.sync.dma_start(out=enc_sb[:], in_=enc_r[s])
                oneh_sb = oneh_tiles[s]
                if s == 0:
                    # kc-outer for the first sample only: start matmuls as
                    # soon as enc chunk 0 lands instead of waiting for the
                    # whole sample (6 concurrent psum groups from ps2).
                    pts = [
                        ps2.tile([P, NV], f32, tag="ps2", name=f"pa{i}")
                        for i in range(KO)
                    ]
                    for kc in range(KC):
                        for ko in range(KO):
                            nc.tensor.matmul(
                                pts[ko][:, :WMAX],
                                lhsT=enc_sb[:, kc, ko * P:(ko + 1) * P],
                                rhs=oneh_sb[:, kc, :],
                                start=(kc == 0),
                                stop=(kc == KC - 1),
                            )
                    load_w(0)
                    for ko in range(KO):
                        nc.vector.tensor_copy(
                            out=mergedT[:, ko, s, :], in_=pts[ko][:, :WMAX]
                        )
                    continue
                if s == 1:
                    load_w(1)
                for ko in range(KO):
                    pt = ps1.tile([P, WMAX], f32, tag="ps1")
                    for kc in range(KC):
                        nc.tensor.matmul(
                            pt[:],
                            lhsT=enc_sb[:, kc, ko * P:(ko + 1) * P],
                            rhs=oneh_sb[:, kc, :],
                            start=(kc == 0),
                            stop=(kc == KC - 1),
                        )
                    nc.vector.tensor_copy(out=mergedT[:, ko, s, :], in_=pt[:])

            # ---- Stage B: out[s, w, v] = mergedT^T @ W, tiled over vocab ----
            # The last vocab chunk is computed in two half-width passes so
            # the final (unoverlappable) psum-copy + store drain is halved.
            for n in range(NT):
                load_w(n + 2)
                w_sb = w_tiles.pop(n)
                segs = [(0, NV)] if n < NT - 1 else [(0, NV // 2), (NV // 2, NV // 2)]
                for c0, cw in segs:
                    for s in range(PB):
                        for wt in range(WT):
                            pt = ps2.tile([P, NV], f32, tag="ps2")
                            for ko in range(KO):
                                nc.tensor.matmul(
                                    pt[:, :cw],
                                    lhsT=mergedT[:, ko, s, wt * P:(wt + 1) * P],
                                    rhs=w_sb[:, ko, c0:c0 + cw],
                                    start=(ko == 0),
                                    stop=(ko == KO - 1),
                                )
                            ot = outp.tile([P, NV], f32, tag="out")
                            nc.vector.tensor_copy(out=ot[:, :cw], in_=pt[:, :cw])
                            nc.sync.dma_start(
                                out=out_d[s, wt * P:(wt + 1) * P,
                                          n * NV + c0:n * NV + c0 + cw],
                                in_=ot[:, :cw],
                            )

    nc.finalize()
    return nc


def _get_program():
    global _compiled
    if _compiled is None:
        _compiled = _build_program()
    return _compiled


def _prep_inputs(bert_encodings, segment_ids, W):
    enc_bf = np.asarray(bert_encodings, dtype=np.float32).astype(ml_dtypes.bfloat16)
    w_bf = np.asarray(W, dtype=np.float32).astype(ml_dtypes.bfloat16)

    ids = np.asarray(segment_ids).astype(np.int64)
    flat = (ids + np.arange(B, dtype=np.int64)[:, None] * WMAX).ravel()
    counts = np.bincount(flat, minlength=B * WMAX).reshape(B, WMAX)
    inv = (1.0 / np.maximum(counts, 1)).astype(np.float32)

    # per-token (segment id, 1/count) pairs, pre-transposed to the SBUF
    # layout [p, sample, {id,inv}, kc] so each core gets one contiguous DMA
    idsval = np.empty((B, 2, S), dtype=np.float32)
    idsval[:, 0, :] = ids.astype(np.float32)
    idsval[:, 1, :] = np.take_along_axis(inv, ids, axis=1)
    aux = np.ascontiguousarray(
        idsval.reshape(NCORES, PB, 2, KC, P).transpose(0, 4, 1, 2, 3)
    )
    return enc_bf, w_bf, aux


def kernel(bert_encodings, segment_ids, W, b, num_words, _trace=False):
    from concourse.bass_utils import run_bass_kernel_spmd

    assert int(num_words) == WMAX
    enc_bf, w_bf, aux = _prep_inputs(bert_encodings, segment_ids, W)

    nc = _get_program()
    core_ids = list(range(NCORES))
    in_maps = [
        {
            "enc": enc_bf[c * PB:(c + 1) * PB],
            "aux": aux[c],
            "wmat": w_bf,
        }
        for c in core_ids
    ]
    res = run_bass_kernel_spmd(nc, in_maps, core_ids, trace=_trace)
    out = np.concatenate([res.results[c]["out"] for c in core_ids], axis=0)
    out = np.ascontiguousarray(out.reshape(B, WMAX, V))

    bias = np.asarray(b, dtype=np.float32)
    if np.any(bias):
        out = out + bias

    if _trace:
        kernel._last_exec_time_ns = res.exec_time_ns
        kernel._last_result = res
    return out



# revision 2
# speedup vs baseline: 1.2218x; 1.2218x over previous
"""BertCorrector kernel for 8 TRN2 NeuronCores.

Computes: segment-mean merge of subword encodings (sorted per-row segment
ids) followed by a dense vocab projection:
    merged[b,w,:] = mean_{s: ids[b,s]==w} enc[b,s,:]   (0 if empty)
    logits = merged @ W + b

Strategy: data-parallel over batch (4 samples/core), with *word
compaction*: only the ~222 non-empty words per sample (of 256 slots) are
computed.  Each sample's words are rank-compacted into a static
WCAP-column block, so stage B's PE time scales with the real word count.

Stage A computes mergedT = enc^T @ S per sample on the TensorEngine,
where S is a one-hot matrix over *compact* word ranks pre-scaled by
1/count (built on-chip from host-side (rank, 1/count) pairs).

Stage B makes W the stationary operand ([128h x 128v] tiles) and streams
the compacted mergedT columns as the moving operand (N = 2*WCAP/...), so
no 128-column padding is wasted on empty words.  Output is written
compacted and transposed ([vocab, words] fp16); the host scatters it
back to the full [B, 256, V] f32 layout.
"""

import numpy as np
import ml_dtypes

B, S, H = 32, 512, 768
V = 8192
WMAX = 256
NCORES = 8
PB = B // NCORES  # samples per core
P = 128

KC = S // P   # 4 token chunks (contraction of stage A)
KO = H // P   # 6 hidden chunks
NVC = V // P  # 64 vocab chunks of 128

_compiled = {}


def _build_program(wcap):
    import concourse.bass as bass
    import concourse.mybir as mybir
    from concourse import bacc
    from concourse.tile import TileContext

    bf16 = mybir.dt.bfloat16
    fp16 = mybir.dt.float16
    f32 = mybir.dt.float32

    cw = PB * wcap      # compact word columns per core
    nh = cw // 2        # moving free dim per matmul (fits one PSUM bank)
    assert nh <= 512

    nc = bacc.Bacc()
    enc_d = nc.dram_tensor("enc", [PB, S, H], bf16, kind="ExternalInput")
    aux_d = nc.dram_tensor("aux", [P, PB, 2, KC], f32, kind="ExternalInput")
    w_d = nc.dram_tensor("wmat", [H, V], bf16, kind="ExternalInput")
    out_d = nc.dram_tensor("out", [NVC, P, cw], fp16, kind="ExternalOutput")

    enc_r = enc_d.rearrange("b (kc p) h -> b p kc h", p=P)
    w_r = w_d.rearrange("(ko p) v -> p ko v", p=P)

    with TileContext(nc) as tc:
        with (
            tc.tile_pool(name="persist", bufs=1) as persist,
            tc.tile_pool(name="encp", bufs=2) as encp,
            tc.tile_pool(name="onehp", bufs=4) as onehp,
            tc.tile_pool(name="wp", bufs=3) as wp,
            tc.tile_pool(name="outp", bufs=6) as outp,
            tc.tile_pool(name="ps1", bufs=2, space="PSUM") as ps1,
            tc.tile_pool(name="ps2", bufs=6, space="PSUM") as ps2,
        ):
            # mergedT[h_in_chunk, ko, compact_word] resident in SBUF (bf16)
            mergedT = persist.tile([P, KO, cw], bf16)

            w_tiles = {}

            def load_w(n):
                if n < NVC:
                    t = wp.tile([P, KO, P], bf16, tag="w")
                    nc.sync.dma_start(out=t[:], in_=w_r[:, :, n * P:(n + 1) * P])
                    w_tiles[n] = t

            # iota row (0..wcap-1, identical on every partition), on-device
            iota_sb = persist.tile([P, wcap], f32)
            nc.gpsimd.iota(
                iota_sb[:], pattern=[[1, wcap]], base=0,
                channel_multiplier=0, allow_small_or_imprecise_dtypes=True,
            )
            # all samples' (compact rank, 1/count) pairs in one contiguous DMA
            aux_sb = persist.tile([P, PB, 2, KC], f32)
            nc.sync.dma_start(out=aux_sb[:], in_=aux_d[:])

            # Warm the PE clock gate while the first input DMAs fly.  Wide
            # moving operand (N=512) keeps the array densely busy so the
            # HAM activity window trips as early as possible.
            warm_sb = persist.tile([P, P + 512], bf16)
            nc.gpsimd.memset(warm_sb[:], 0.0)
            warm_ps = ps1.tile([P, 512], f32, tag="ps1")
            for _ in range(18):
                nc.tensor.matmul(
                    warm_ps[:], lhsT=warm_sb[:, :P], rhs=warm_sb[:, P:],
                    start=True, stop=True,
                )

            # ---- Stage A: mergedT = enc^T @ scaled_onehot, per sample ----
            # Scaled one-hots for ALL samples up front (DVE runs in
            # emission order; these must not queue behind mergedT casts):
            # oneh[tok, r] = (iota[r] == rank[tok]) / count
            oneh_tiles = []
            for s in range(PB):
                t = onehp.tile([P, KC, wcap], bf16, tag="oneh", name=f"oneh{s}")
                for kc in range(KC):
                    nc.vector.tensor_scalar(
                        out=t[:, kc],
                        in0=iota_sb[:],
                        scalar1=aux_sb[:, s, 0, kc:kc + 1],
                        scalar2=aux_sb[:, s, 1, kc:kc + 1],
                        op0=mybir.AluOpType.is_equal,
                        op1=mybir.AluOpType.mult,
                    )
                oneh_tiles.append(t)

            for s in range(PB):
                enc_sb = encp.tile([P, KC, H], bf16, tag="enc")
                if s == 0:
                    # split across DMA queues so the first chunk lands sooner
                    for kc in range(KC):
                        nc.sync.dma_start(out=enc_sb[:, kc], in_=enc_r[s, :, kc])
                else:
                    nc.sync.dma_start(out=enc_sb[:], in_=enc_r[s])
                oneh_sb = oneh_tiles[s]
                if s == 0:
                    # kc-outer for the first sample only: start matmuls as
                    # soon as enc chunk 0 lands instead of waiting for the
                    # whole sample (6 concurrent psum groups from ps2).
                    pts = [
                        ps2.tile([P, wcap], f32, tag="ps2", name=f"pa{i}")
                        for i in range(KO)
                    ]
                    for kc in range(KC):
                        for ko in range(KO):
                            nc.tensor.matmul(
                                pts[ko][:],
                                lhsT=enc_sb[:, kc, ko * P:(ko + 1) * P],
                                rhs=oneh_sb[:, kc, :],
                                start=(kc == 0),
                                stop=(kc == KC - 1),
                            )
                    load_w(0)
                    for ko in range(KO):
                        nc.vector.tensor_copy(
                            out=mergedT[:, ko, s * wcap:(s + 1) * wcap],
                            in_=pts[ko][:],
                        )
                    continue
                if s == 1:
                    load_w(1)
                for ko in range(KO):
                    pt = ps1.tile([P, wcap], f32, tag="ps1")
                    for kc in range(KC):
                        nc.tensor.matmul(
                            pt[:],
                            lhsT=enc_sb[:, kc, ko * P:(ko + 1) * P],
                            rhs=oneh_sb[:, kc, :],
                            start=(kc == 0),
                            stop=(kc == KC - 1),
                        )
                    nc.vector.tensor_copy(
                        out=mergedT[:, ko, s * wcap:(s + 1) * wcap], in_=pt[:]
                    )

            # ---- Stage B: out[v, w] = W^T @ mergedT, tiled over vocab ----
            # W tile [128h, 128v] is stationary; the compacted word columns
            # stream as the moving operand in two PSUM-bank halves.
            for n in range(NVC):
                load_w(n + 2)
                w_sb = w_tiles.pop(n)
                pt0 = ps2.tile([P, nh], f32, tag="ps2")
                pt1 = ps2.tile([P, nh], f32, tag="ps2")
                for ko in range(KO):
                    lhsT = w_sb[:, ko, :]
                    nc.tensor.matmul(
                        pt0[:], lhsT=lhsT, rhs=mergedT[:, ko, 0:nh],
                        start=(ko == 0), stop=(ko == KO - 1),
                    )
                    nc.tensor.matmul(
                        pt1[:], lhsT=lhsT, rhs=mergedT[:, ko, nh:cw],
                        start=(ko == 0), stop=(ko == KO - 1),
                    )
                ot = outp.tile([P, cw], fp16, tag="out")
                nc.vector.tensor_copy(out=ot[:, :nh], in_=pt0[:])
                nc.vector.tensor_copy(out=ot[:, nh:], in_=pt1[:])
                nc.sync.dma_start(out=out_d[n], in_=ot[:])

    nc.finalize()
    return nc


def _get_program(wcap):
    if wcap not in _compiled:
        _compiled[wcap] = _build_program(wcap)
    return _compiled[wcap]


def _prep_inputs(bert_encodings, segment_ids, W, wcap):
    enc_bf = np.asarray(bert_encodings, dtype=np.float32).astype(ml_dtypes.bfloat16)
    w_bf = np.asarray(W, dtype=np.float32).astype(ml_dtypes.bfloat16)

    ids = np.asarray(segment_ids).astype(np.int64)
    # Per-sample compact rank of each token's word + 1/count, plus the
    # sorted unique word list for the host-side scatter.
    comp = np.empty((B, S), dtype=np.float32)
    inv = np.empty((B, S), dtype=np.float32)
    word_lists = []
    for b in range(B):
        u, idx, cnt = np.unique(ids[b], return_inverse=True, return_counts=True)
        comp[b] = idx.astype(np.float32)
        inv[b] = (1.0 / cnt[idx]).astype(np.float32)
        word_lists.append(u)

    idsval = np.empty((B, 2, S), dtype=np.float32)
    idsval[:, 0, :] = comp
    idsval[:, 1, :] = inv
    # pre-transposed to the SBUF layout [p, sample, {rank,inv}, kc] so each
    # core gets one contiguous DMA
    aux = np.ascontiguousarray(
        idsval.reshape(NCORES, PB, 2, KC, P).transpose(0, 4, 1, 2, 3)
    )
    return enc_bf, w_bf, aux, word_lists


def kernel(bert_encodings, segment_ids, W, b, num_words, _trace=False):
    from concourse.bass_utils import run_bass_kernel_spmd

    assert int(num_words) == WMAX
    ids = np.asarray(segment_ids)
    max_nnz = max(len(np.unique(ids[i])) for i in range(B))
    wcap = 232 if max_nnz <= 232 else WMAX

    enc_bf, w_bf, aux, word_lists = _prep_inputs(bert_encodings, ids, W, wcap)

    nc = _get_program(wcap)
    core_ids = list(range(NCORES))
    in_maps = [
        {
            "enc": enc_bf[c * PB:(c + 1) * PB],
            "aux": aux[c],
            "wmat": w_bf,
        }
        for c in core_ids
    ]
    res = run_bass_kernel_spmd(nc, in_maps, core_ids, trace=_trace)

    out = np.zeros((B, WMAX, V), dtype=np.float32)
    cw = PB * wcap
    for c in core_ids:
        # [NVC, P, cw] fp16 -> [V, cw] -> f32 -> [cw, V]
        flat = np.ascontiguousarray(
            np.asarray(res.results[c]["out"]).reshape(V, cw).astype(np.float32).T
        )
        for s in range(PB):
            bi = c * PB + s
            u = word_lists[bi]
            out[bi, u, :] = flat[s * wcap:s * wcap + len(u)]

    bias = np.asarray(b, dtype=np.float32)
    if np.any(bias):
        out = out + bias

    if _trace:
        kernel._last_exec_time_ns = res.exec_time_ns
        kernel._last_result = res
    return out


# revision 3
# speedup vs baseline: 1.2794x; 1.0471x over previous
"""BertCorrector kernel for 8 TRN2 NeuronCores.

Computes: segment-mean merge of subword encodings (sorted per-row segment
ids) followed by a dense vocab projection:
    merged[b,w,:] = mean_{s: ids[b,s]==w} enc[b,s,:]   (0 if empty)
    logits = merged @ W + b

Strategy: data-parallel over batch (4 samples/core), with *word
compaction*: only the non-empty words per sample (of 256 slots) are
computed.  Each sample's words are rank-compacted into a static
per-slot column block, and samples are assigned to slots sorted by
word count so the slot capacities hug the actual counts.

Stage A computes mergedT = enc^T @ S per sample on the TensorEngine,
where S is a one-hot matrix over *compact* word ranks pre-scaled by
1/count (built on-chip from host-side (rank, 1/count) pairs).  Stage A
PSUM->SBUF copies run on the Scalar (ACT) engine so they never queue
behind the one-hot builds on Vector.

Stage B makes W the stationary operand ([128h x 128v] tiles) and
streams the compacted mergedT columns as the moving operand, so PE time
scales with the real word count instead of the padded 256/sample.
Output is written compacted and transposed ([vocab, words] fp16); the
host scatters it back to the full [B, 256, V] f32 layout.
"""

import numpy as np
import ml_dtypes

B, S, H = 32, 512, 768
V = 8192
WMAX = 256
NCORES = 8
PB = B // NCORES  # samples per core
P = 128

KC = S // P   # 4 token chunks (contraction of stage A)
KO = H // P   # 6 hidden chunks
NVC = V // P  # 64 vocab chunks of 128

# Per-slot word capacities (samples assigned to slots by descending word
# count, so slot k's capacity only has to cover the k-th octile).
CAPS_FAST = (230, 227, 223, 220)
CAPS_FULL = (WMAX,) * PB

_compiled = {}


def _build_program(caps):
    import concourse.bass as bass
    import concourse.mybir as mybir
    from concourse import bacc
    from concourse.tile import TileContext

    bf16 = mybir.dt.bfloat16
    fp16 = mybir.dt.float16
    f32 = mybir.dt.float32

    offs = [0]
    for c in caps:
        offs.append(offs[-1] + c)
    cw = offs[-1]                 # compact word columns per core
    n0 = caps[0] + caps[1]        # moving free dim, first PSUM half
    n1 = caps[2] + caps[3]        # second half
    cmax = max(caps)
    assert n0 <= 512 and n1 <= 512

    nc = bacc.Bacc()
    enc_d = nc.dram_tensor("enc", [PB, S, H], bf16, kind="ExternalInput")
    aux_d = nc.dram_tensor("aux", [P, PB, 2, KC], f32, kind="ExternalInput")
    w_d = nc.dram_tensor("wmat", [H, V], bf16, kind="ExternalInput")
    out_d = nc.dram_tensor("out", [NVC, P, cw], fp16, kind="ExternalOutput")

    enc_r = enc_d.rearrange("b (kc p) h -> b p kc h", p=P)
    w_r = w_d.rearrange("(ko p) v -> p ko v", p=P)
    out_r = out_d.rearrange("(n2 two) p w -> n2 p two w", two=2)

    with TileContext(nc) as tc:
        with (
            tc.tile_pool(name="persist", bufs=1) as persist,
            tc.tile_pool(name="encp", bufs=4) as encp,
            tc.tile_pool(name="onehp", bufs=4) as onehp,
            tc.tile_pool(name="wp", bufs=4) as wp,
            tc.tile_pool(name="outp", bufs=3) as outp,
            tc.tile_pool(name="ps1", bufs=2, space="PSUM") as ps1,
            tc.tile_pool(name="ps2", bufs=6, space="PSUM") as ps2,
        ):
            # mergedT[h_in_chunk, ko, compact_word] resident in SBUF (bf16)
            mergedT = persist.tile([P, KO, cw], bf16)

            w_tiles = {}

            def load_wpair(n2):
                if n2 < NVC // 2:
                    t = wp.tile([P, KO, 2 * P], bf16, tag="w")
                    nc.sync.dma_start(
                        out=t[:], in_=w_r[:, :, n2 * 2 * P:(n2 + 1) * 2 * P]
                    )
                    w_tiles[n2] = t

            # iota row (0..cmax-1, identical on every partition), on-device
            iota_sb = persist.tile([P, cmax], f32)
            nc.gpsimd.iota(
                iota_sb[:], pattern=[[1, cmax]], base=0,
                channel_multiplier=0, allow_small_or_imprecise_dtypes=True,
            )
            # all slots' (compact rank, 1/count) pairs in one contiguous DMA
            aux_sb = persist.tile([P, PB, 2, KC], f32)
            nc.sync.dma_start(out=aux_sb[:], in_=aux_d[:])

            # Scaled one-hots for ALL slots up front on Vector (stage-A
            # copies run on Scalar, so they never contend):
            # oneh[tok, r] = (iota[r] == rank[tok]) / count
            oneh_tiles = []
            for s in range(PB):
                t = onehp.tile([P, KC, caps[s]], bf16, tag="oneh", name=f"oneh{s}")
                for kc in range(KC):
                    nc.vector.tensor_scalar(
                        out=t[:, kc],
                        in0=iota_sb[:, :caps[s]],
                        scalar1=aux_sb[:, s, 0, kc:kc + 1],
                        scalar2=aux_sb[:, s, 1, kc:kc + 1],
                        op0=mybir.AluOpType.is_equal,
                        op1=mybir.AluOpType.mult,
                    )
                oneh_tiles.append(t)

            # ---- Stage A: mergedT = enc^T @ scaled_onehot, per sample ----
            # No PE warmup: stage A's dense matmul stream trips the HAM
            # clock gate itself well before stage B begins.
            for s in range(PB):
                enc_sb = encp.tile([P, KC, H], bf16, tag="enc")
                if s == 0:
                    # split across DMA queues so the first chunk lands sooner
                    for kc in range(KC):
                        nc.sync.dma_start(out=enc_sb[:, kc], in_=enc_r[s, :, kc])
                else:
                    nc.sync.dma_start(out=enc_sb[:], in_=enc_r[s])
                oneh_sb = oneh_tiles[s]
                if s == 0:
                    # kc-outer for the first sample only: start matmuls as
                    # soon as enc chunk 0 lands instead of waiting for the
                    # whole sample (6 concurrent psum groups from ps2).
                    pts = [
                        ps2.tile([P, caps[0]], f32, tag="ps2", name=f"pa{i}")
                        for i in range(KO)
                    ]
                    for kc in range(KC):
                        for ko in range(KO):
                            nc.tensor.matmul(
                                pts[ko][:],
                                lhsT=enc_sb[:, kc, ko * P:(ko + 1) * P],
                                rhs=oneh_sb[:, kc, :],
                                start=(kc == 0),
                                stop=(kc == KC - 1),
                            )
                    load_wpair(0)
                    for ko in range(KO):
                        nc.scalar.copy(
                            out=mergedT[:, ko, offs[0]:offs[1]], in_=pts[ko][:]
                        )
                    continue
                if s == 1:
                    load_wpair(1)
                if s == 2:
                    load_wpair(2)
                for ko in range(KO):
                    pt = ps1.tile([P, caps[s]], f32, tag="ps1")
                    for kc in range(KC):
                        nc.tensor.matmul(
                            pt[:],
                            lhsT=enc_sb[:, kc, ko * P:(ko + 1) * P],
                            rhs=oneh_sb[:, kc, :],
                            start=(kc == 0),
                            stop=(kc == KC - 1),
                        )
                    nc.scalar.copy(
                        out=mergedT[:, ko, offs[s]:offs[s + 1]], in_=pt[:]
                    )

            # ---- Stage B: out[v, w] = W^T @ mergedT, tiled over vocab ----
            # W tile [128h, 128v] is stationary; the compacted word columns
            # stream as the moving operand in two PSUM-bank halves.  Output
            # DMAs are batched two vocab chunks at a time; the final chunk
            # is split into per-slot quarters to shorten the drain.
            for n2 in range(NVC // 2):
                load_wpair(n2 + 3)
                w_sb = w_tiles.pop(n2)
                last = n2 == NVC // 2 - 1
                ot = outp.tile([P, 2, cw], fp16, tag="out")
                for j in range(2):
                    if last and j == 1:
                        break
                    pt0 = ps2.tile([P, n0], f32, tag="ps2")
                    pt1 = ps2.tile([P, n1], f32, tag="ps2")
                    for ko in range(KO):
                        lhsT = w_sb[:, ko, j * P:(j + 1) * P]
                        nc.tensor.matmul(
                            pt0[:], lhsT=lhsT, rhs=mergedT[:, ko, 0:n0],
                            start=(ko == 0), stop=(ko == KO - 1),
                        )
                        nc.tensor.matmul(
                            pt1[:], lhsT=lhsT, rhs=mergedT[:, ko, n0:cw],
                            start=(ko == 0), stop=(ko == KO - 1),
                        )
                    nc.vector.tensor_copy(out=ot[:, j, :n0], in_=pt0[:])
                    nc.scalar.copy(out=ot[:, j, n0:], in_=pt1[:])
                if not last:
                    nc.sync.dma_start(out=out_r[n2], in_=ot[:])
                else:
                    nc.sync.dma_start(out=out_d[2 * n2], in_=ot[:, 0])
                    # final vocab chunk: per-slot quarter chains so copy +
                    # store overlap the tail matmuls
                    for s in range(PB):
                        pq = ps2.tile([P, caps[s]], f32, tag="ps2")
                        for ko in range(KO):
                            nc.tensor.matmul(
                                pq[:],
                                lhsT=w_sb[:, ko, P:2 * P],
                                rhs=mergedT[:, ko, offs[s]:offs[s + 1]],
                                start=(ko == 0), stop=(ko == KO - 1),
                            )
                        oq = outp.tile([P, caps[s]], fp16, tag="outq")
                        if s % 2 == 0:
                            nc.vector.tensor_copy(out=oq[:], in_=pq[:])
                        else:
                            nc.scalar.copy(out=oq[:], in_=pq[:])
                        nc.sync.dma_start(
                            out=out_d[2 * n2 + 1, :, offs[s]:offs[s + 1]],
                            in_=oq[:],
                        )

    nc.finalize()
    return nc


def _get_program(caps):
    if caps not in _compiled:
        _compiled[caps] = _build_program(caps)
    return _compiled[caps]


def _prep_inputs(bert_encodings, segment_ids, W):
    enc_bf = np.asarray(bert_encodings, dtype=np.float32).astype(ml_dtypes.bfloat16)
    w_bf = np.asarray(W, dtype=np.float32).astype(ml_dtypes.bfloat16)

    ids = np.asarray(segment_ids).astype(np.int64)
    uniq = []   # per sample: sorted unique word ids
    comp = np.empty((B, S), dtype=np.float32)
    inv = np.empty((B, S), dtype=np.float32)
    for b in range(B):
        u, idx, cnt = np.unique(ids[b], return_inverse=True, return_counts=True)
        uniq.append(u)
        comp[b] = idx.astype(np.float32)
        inv[b] = (1.0 / cnt[idx]).astype(np.float32)
    nnz = np.array([len(u) for u in uniq])

    # slot assignment: rank samples by descending word count; slot k of
    # core c takes rank k*NCORES + c
    order = np.argsort(-nnz, kind="stable")
    perm = order.reshape(PB, NCORES).T  # [core, slot] -> sample
    caps = CAPS_FAST
    for k in range(PB):
        if nnz[perm[:, k]].max() > caps[k]:
            caps = CAPS_FULL
            break

    # per-token (compact rank, 1/count), transposed to the SBUF layout
    # [p, slot, {rank,inv}, kc] so each core gets one contiguous DMA
    aux = np.empty((NCORES, P, PB, 2, KC), dtype=np.float32)
    for c in range(NCORES):
        for k in range(PB):
            b = perm[c, k]
            aux[c, :, k, 0, :] = comp[b].reshape(KC, P).T
            aux[c, :, k, 1, :] = inv[b].reshape(KC, P).T
    return enc_bf, w_bf, np.ascontiguousarray(aux), uniq, perm, caps


def kernel(bert_encodings, segment_ids, W, b, num_words, _trace=False):
    from concourse.bass_utils import run_bass_kernel_spmd

    assert int(num_words) == WMAX
    enc_bf, w_bf, aux, uniq, perm, caps = _prep_inputs(bert_encodings, segment_ids, W)

    offs = [0]
    for c in caps:
        offs.append(offs[-1] + c)
    cw = offs[-1]

    nc = _get_program(caps)
    core_ids = list(range(NCORES))
    in_maps = [
        {
            "enc": np.ascontiguousarray(enc_bf[perm[c]]),
            "aux": aux[c],
            "wmat": w_bf,
        }
        for c in core_ids
    ]
    res = run_bass_kernel_spmd(nc, in_maps, core_ids, trace=_trace)

    out = np.zeros((B, WMAX, V), dtype=np.float32)
    for c in core_ids:
        # [NVC, P, cw] fp16 -> [V, cw] -> f32 -> [cw, V]
        flat = np.ascontiguousarray(
            np.asarray(res.results[c]["out"]).reshape(V, cw).astype(np.float32).T
        )
        for s in range(PB):
            bi = perm[c, s]
            u = uniq[bi]
            out[bi, u, :] = flat[offs[s]:offs[s] + len(u)]

    bias = np.asarray(b, dtype=np.float32)
    if np.any(bias):
        out = out + bias

    if _trace:
        kernel._last_exec_time_ns = res.exec_time_ns
        kernel._last_result = res
    return out


# revision 8
# speedup vs baseline: 1.3042x; 1.0194x over previous
"""BertCorrector kernel for 8 TRN2 NeuronCores.

Computes: segment-mean merge of subword encodings (sorted per-row segment
ids) followed by a dense vocab projection:
    merged[b,w,:] = mean_{s: ids[b,s]==w} enc[b,s,:]   (0 if empty)
    logits = merged @ W + b

Strategy: data-parallel over batch (4 samples/core), with *word
compaction*: only the non-empty words per sample (of 256 slots) are
computed.  Each sample's words are rank-compacted into a static
per-slot column block, and samples are assigned to slots sorted by
word count so the slot capacities hug the actual counts.

Stage A computes mergedT = enc^T @ S per sample on the TensorEngine,
where S is a one-hot matrix over *compact* word ranks pre-scaled by
1/count (built on-chip from host-side (rank, 1/count) pairs).  Stage A
PSUM->SBUF copies run on the Scalar (ACT) engine so they never queue
behind the one-hot builds on Vector.

Stage B makes W the stationary operand ([128h x 128v] tiles) and
streams the compacted mergedT columns as the moving operand, so PE time
scales with the real word count instead of the padded 256/sample.
Output is written compacted and transposed ([vocab, words] fp16); the
host scatters it back to the full [B, 256, V] f32 layout.
"""

import numpy as np
import ml_dtypes

B, S, H = 32, 512, 768
V = 8192
WMAX = 256
NCORES = 8
PB = B // NCORES  # samples per core
P = 128

KC = S // P   # 4 token chunks (contraction of stage A)
KO = H // P   # 6 hidden chunks
NVC = V // P  # 64 vocab chunks of 128

# Per-slot word capacities (samples assigned to slots by descending word
# count, so slot k's capacity only has to cover the k-th octile).
CAPS_FAST = (230, 227, 223, 220)
CAPS_FULL = (WMAX,) * PB

_compiled = {}


def _build_program(caps):
    import concourse.bass as bass
    import concourse.mybir as mybir
    from concourse import bacc
    from concourse.tile import TileContext

    bf16 = mybir.dt.bfloat16
    fp16 = mybir.dt.float16
    f32 = mybir.dt.float32

    offs = [0]
    for c in caps:
        offs.append(offs[-1] + c)
    cw = offs[-1]                 # compact word columns per core
    n0 = caps[0] + caps[1]        # moving free dim, first PSUM half
    n1 = caps[2] + caps[3]        # second half
    cmax = max(caps)
    assert n0 <= 512 and n1 <= 512

    nc = bacc.Bacc()
    enc_d = nc.dram_tensor("enc", [PB, S, H], bf16, kind="ExternalInput")
    aux_d = nc.dram_tensor("aux", [P, PB, 2, KC], f32, kind="ExternalInput")
    w_d = nc.dram_tensor("wmat", [H, V], bf16, kind="ExternalInput")
    out_d = nc.dram_tensor("out", [NVC, P, cw], fp16, kind="ExternalOutput")

    enc_r = enc_d.rearrange("b (kc p) h -> b p kc h", p=P)
    w_r = w_d.rearrange("(ko p) v -> p ko v", p=P)
    out_r = out_d.rearrange("(n2 two) p w -> n2 p two w", two=2)

    with TileContext(nc) as tc:
        with (
            tc.tile_pool(name="persist", bufs=1) as persist,
            tc.tile_pool(name="encp", bufs=4) as encp,
            tc.tile_pool(name="onehp", bufs=4) as onehp,
            tc.tile_pool(name="wp", bufs=4) as wp,
            tc.tile_pool(name="outp", bufs=3) as outp,
            tc.tile_pool(name="outq", bufs=2) as outq,
            tc.tile_pool(name="ps1", bufs=2, space="PSUM") as ps1,
            tc.tile_pool(name="ps2", bufs=6, space="PSUM") as ps2,
        ):
            # mergedT[h_in_chunk, ko, compact_word] resident in SBUF (bf16)
            mergedT = persist.tile([P, KO, cw], bf16)

            w_tiles = {}

            def load_wpair(n2):
                if n2 < NVC // 2:
                    t = wp.tile([P, KO, 2 * P], bf16, tag="w")
                    nc.sync.dma_start(
                        out=t[:], in_=w_r[:, :, n2 * 2 * P:(n2 + 1) * 2 * P]
                    )
                    w_tiles[n2] = t

            # Warmup operands first on the GpSimd queue so the PE clock
            # gate can start ramping before iota/one-hot are ready.
            warm_sb = persist.tile([P, P + 512], bf16)
            nc.gpsimd.memset(warm_sb[:], 0.0)
            # iota row (0..cmax-1, identical on every partition), on-device
            iota_sb = persist.tile([P, cmax], f32)
            nc.gpsimd.iota(
                iota_sb[:], pattern=[[1, cmax]], base=0,
                channel_multiplier=0, allow_small_or_imprecise_dtypes=True,
            )
            # all slots' (compact rank, 1/count) pairs in one contiguous DMA
            aux_sb = persist.tile([P, PB, 2, KC], f32)
            nc.sync.dma_start(out=aux_sb[:], in_=aux_d[:])

            # Dense warmup (shared stationary, N=512) trips the HAM clock
            # gate during the enc-DMA/one-hot latency so stage A runs at
            # the full 2.4 GHz PE clock.
            warm_ps = ps1.tile([P, 512], f32, tag="ps1")
            for _ in range(10):
                nc.tensor.matmul(
                    warm_ps[:], lhsT=warm_sb[:, :P], rhs=warm_sb[:, P:],
                    start=True, stop=True,
                )

            # Scaled one-hots for ALL slots up front on Vector (stage-A
            # copies run on Scalar, so they never contend):
            # oneh[tok, r] = (iota[r] == rank[tok]) / count
            oneh_tiles = []
            for s in range(PB):
                t = onehp.tile([P, KC, caps[s]], bf16, tag="oneh", name=f"oneh{s}")
                for kc in range(KC):
                    nc.vector.tensor_scalar(
                        out=t[:, kc],
                        in0=iota_sb[:, :caps[s]],
                        scalar1=aux_sb[:, s, 0, kc:kc + 1],
                        scalar2=aux_sb[:, s, 1, kc:kc + 1],
                        op0=mybir.AluOpType.is_equal,
                        op1=mybir.AluOpType.mult,
                    )
                oneh_tiles.append(t)

            # ---- Stage A: mergedT = enc^T @ scaled_onehot, per sample ----
            # All enc DMAs are emitted before any W load so the saturated
            # startup DMA window services stage A's inputs first.
            enc_tiles = []
            for s in range(PB):
                enc_sb = encp.tile([P, KC, H], bf16, tag="enc")
                if s == 0:
                    # split across DMA queues so the first chunk lands sooner
                    for kc in range(KC):
                        nc.sync.dma_start(out=enc_sb[:, kc], in_=enc_r[s, :, kc])
                else:
                    nc.sync.dma_start(out=enc_sb[:], in_=enc_r[s])
                enc_tiles.append(enc_sb)

            for s in range(PB):
                enc_sb = enc_tiles[s]
                oneh_sb = oneh_tiles[s]
                if s == 0:
                    # kc-outer for the first sample only: start matmuls as
                    # soon as enc chunk 0 lands instead of waiting for the
                    # whole sample (6 concurrent psum groups from ps2).
                    pts = [
                        ps2.tile([P, caps[0]], f32, tag="ps2", name=f"pa{i}")
                        for i in range(KO)
                    ]
                    for kc in range(KC):
                        for ko in range(KO):
                            nc.tensor.matmul(
                                pts[ko][:],
                                lhsT=enc_sb[:, kc, ko * P:(ko + 1) * P],
                                rhs=oneh_sb[:, kc, :],
                                start=(kc == 0),
                                stop=(kc == KC - 1),
                            )
                    for ko in range(KO):
                        nc.scalar.copy(
                            out=mergedT[:, ko, offs[0]:offs[1]], in_=pts[ko][:]
                        )
                    continue
                if s == PB - 1:
                    # W prefetch only after every enc DMA is in the queue
                    load_wpair(0)
                    load_wpair(1)
                    load_wpair(2)
                for ko in range(KO):
                    pt = ps1.tile([P, caps[s]], f32, tag="ps1")
                    for kc in range(KC):
                        nc.tensor.matmul(
                            pt[:],
                            lhsT=enc_sb[:, kc, ko * P:(ko + 1) * P],
                            rhs=oneh_sb[:, kc, :],
                            start=(kc == 0),
                            stop=(kc == KC - 1),
                        )
                    nc.scalar.copy(
                        out=mergedT[:, ko, offs[s]:offs[s + 1]], in_=pt[:]
                    )

            # ---- Stage B: out[v, w] = W^T @ mergedT, tiled over vocab ----
            # W tile [128h, 128v] is stationary; the compacted word columns
            # stream as the moving operand in two PSUM-bank halves.  Output
            # DMAs are batched two vocab chunks at a time; the final chunk
            # is split into per-slot quarters to shorten the drain.
            for n2 in range(NVC // 2):
                load_wpair(n2 + 3)
                w_sb = w_tiles.pop(n2)
                last = n2 == NVC // 2 - 1
                ot = outp.tile([P, 2, cw], fp16, tag="out")
                for j in range(2):
                    if last and j == 1:
                        break
                    pt0 = ps2.tile([P, n0], f32, tag="ps2")
                    pt1 = ps2.tile([P, n1], f32, tag="ps2")
                    for ko in range(KO):
                        lhsT = w_sb[:, ko, j * P:(j + 1) * P]
                        nc.tensor.matmul(
                            pt0[:], lhsT=lhsT, rhs=mergedT[:, ko, 0:n0],
                            start=(ko == 0), stop=(ko == KO - 1),
                        )
                        nc.tensor.matmul(
                            pt1[:], lhsT=lhsT, rhs=mergedT[:, ko, n0:cw],
                            start=(ko == 0), stop=(ko == KO - 1),
                        )
                    nc.vector.tensor_copy(out=ot[:, j, :n0], in_=pt0[:])
                    nc.scalar.copy(out=ot[:, j, n0:], in_=pt1[:])
                if not last:
                    nc.sync.dma_start(out=out_r[n2], in_=ot[:])
                else:
                    nc.sync.dma_start(out=out_d[2 * n2], in_=ot[:, 0])
                    # final vocab chunk: per-slot quarter chains so copy +
                    # store overlap the tail matmuls; two batched stores,
                    # the last copy on Vector (its queue drains first)
                    oq = [
                        outq.tile([P, n0], fp16, tag="oq", name="oq0"),
                        outq.tile([P, n1], fp16, tag="oq", name="oq1"),
                    ]
                    for s in range(PB):
                        pq = ps2.tile([P, caps[s]], f32, tag="ps2")
                        for ko in range(KO):
                            nc.tensor.matmul(
                                pq[:],
                                lhsT=w_sb[:, ko, P:2 * P],
                                rhs=mergedT[:, ko, offs[s]:offs[s + 1]],
                                start=(ko == 0), stop=(ko == KO - 1),
                            )
                        half = s // 2
                        lo = offs[s] - offs[half * 2]
                        dst = oq[half][:, lo:lo + caps[s]]
                        if s % 2 == 0:
                            nc.scalar.copy(out=dst, in_=pq[:])
                        else:
                            nc.vector.tensor_copy(out=dst, in_=pq[:])
                        if s % 2 == 1:
                            nc.sync.dma_start(
                                out=out_d[2 * n2 + 1, :,
                                          offs[half * 2]:offs[half * 2 + 2]],
                                in_=oq[half][:],
                            )

    nc.finalize()
    return nc


def _get_program(caps):
    if caps not in _compiled:
        _compiled[caps] = _build_program(caps)
    return _compiled[caps]


def _prep_inputs(bert_encodings, segment_ids, W):
    enc_bf = np.asarray(bert_encodings, dtype=np.float32).astype(ml_dtypes.bfloat16)
    w_bf = np.asarray(W, dtype=np.float32).astype(ml_dtypes.bfloat16)

    ids = np.asarray(segment_ids).astype(np.int64)
    uniq = []   # per sample: sorted unique word ids
    comp = np.empty((B, S), dtype=np.float32)
    inv = np.empty((B, S), dtype=np.float32)
    for b in range(B):
        u, idx, cnt = np.unique(ids[b], return_inverse=True, return_counts=True)
        uniq.append(u)
        comp[b] = idx.astype(np.float32)
        inv[b] = (1.0 / cnt[idx]).astype(np.float32)
    nnz = np.array([len(u) for u in uniq])

    # slot assignment: rank samples by descending word count; slot k of
    # core c takes rank k*NCORES + c
    order = np.argsort(-nnz, kind="stable")
    perm = order.reshape(PB, NCORES).T  # [core, slot] -> sample
    caps = CAPS_FAST
    for k in range(PB):
        if nnz[perm[:, k]].max() > caps[k]:
            caps = CAPS_FULL
            break

    # per-token (compact rank, 1/count), transposed to the SBUF layout
    # [p, slot, {rank,inv}, kc] so each core gets one contiguous DMA
    aux = np.empty((NCORES, P, PB, 2, KC), dtype=np.float32)
    for c in range(NCORES):
        for k in range(PB):
            b = perm[c, k]
            aux[c, :, k, 0, :] = comp[b].reshape(KC, P).T
            aux[c, :, k, 1, :] = inv[b].reshape(KC, P).T
    return enc_bf, w_bf, np.ascontiguousarray(aux), uniq, perm, caps


def kernel(bert_encodings, segment_ids, W, b, num_words, _trace=False):
    from concourse.bass_utils import run_bass_kernel_spmd

    assert int(num_words) == WMAX
    enc_bf, w_bf, aux, uniq, perm, caps = _prep_inputs(bert_encodings, segment_ids, W)

    offs = [0]
    for c in caps:
        offs.append(offs[-1] + c)
    cw = offs[-1]

    nc = _get_program(caps)
    core_ids = list(range(NCORES))
    in_maps = [
        {
            "enc": np.ascontiguousarray(enc_bf[perm[c]]),
            "aux": aux[c],
            "wmat": w_bf,
        }
        for c in core_ids
    ]
    res = run_bass_kernel_spmd(nc, in_maps, core_ids, trace=_trace)

    out = np.zeros((B, WMAX, V), dtype=np.float32)
    for c in core_ids:
        # [NVC, P, cw] fp16 -> [V, cw] -> f32 -> [cw, V]
        flat = np.ascontiguousarray(
            np.asarray(res.results[c]["out"]).reshape(V, cw).astype(np.float32).T
        )
        for s in range(PB):
            bi = perm[c, s]
            u = uniq[bi]
            out[bi, u, :] = flat[offs[s]:offs[s] + len(u)]

    bias = np.asarray(b, dtype=np.float32)
    if np.any(bias):
        out = out + bias

    if _trace:
        kernel._last_exec_time_ns = res.exec_time_ns
        kernel._last_result = res
    return out
